# revision 2
# baseline (speedup 1.0000x reference)
"""AdptWeightBCEDiceLoss Trainium2 kernel, v2 (w-pool-first redesign).

Full inputs y_pred/y_target [32,1,512,512] f32 -> scalar f32 loss.
8 cores x 4 images, data parallel.

Per-core pipeline (half-image units j=0..7, image i=j//2), with t5 = 5t:
  DMA:  s1_i = t5 padded rows [128,4,560] bf16 ([32:544) data), Pf_i bf16
  DVE:  fused 4-row scan -> sc_i (31-wide running w-boxsum, bf16)
  PE:   h-pool band matmul (bb = band/961) + (-I)*t5 accumulate
        -> D'_j = 5*avgpool - t5 in psum f32 [128,2,512]
  ACT:  q_j = |D'_j| -> SBUF bf16, accum -> sq_j  (abs, sigmoid-set filler)
  ACT:  F_j = sigmoid(-Pf_j), accum -> sF_j
  DVE:  ABS_AMR custom op: u_j = (|D'_j|+1)*t5, accum -> su5_j
  GpS:  qF_j = q_j * F_j
  PE:   colsum matmuls (ones stationary) over qF_j -> per-image psum row
  DVE:  TTR custom op: (u_j*F_j) accum -> sx5_j
  PE:   trace matmuls diag(Pf_i^T t5_i) -> zpsA/zpsB per image
  ACT:  phase 2: Ln(F) pairs, accum -> global sum ln F; psum evacs (Copy)
Host combines in float64:
  A = N + sq, sv = sF + colsum(qF), B = (su5-sx5)/5, C = A - sv + su5/5,
  bce = (-sum lnF - sum(x*t5)/5) / (32*N), w_iou = 1-(B+1+eps)/(C-B+1+eps),
  loss = mean(bce*~1 + w_iou)   [w_bce ~= bce; eps-correction negligible]
"""

import numpy as np

import concourse.bacc as bacc
import concourse.bass as bass
import concourse.tile as tile
from concourse import mybir
from concourse.bass_utils import run_bass_kernel_spmd

F32 = mybir.dt.float32
BF16 = mybir.dt.bfloat16

H = W = 512
RB = 4
KPOOL = 31
PADL = 32            # left pad (31 zeros needed; 32 keeps 4B alignment)
SROW = 560           # padded row: [0:32) zeros | [32:544) data | [544:560) zeros
NPIX = H * W
N_CORES = 8
IMG_PER_CORE = 4
NHALF = 2 * IMG_PER_CORE
SMOOTH = 1e-8
QSCALE = 1.0 / (KPOOL * KPOOL)


def register_custom_ops():
    """Register ABS_AFFINE_MUL_REDUCE: out=(|in0|*s0+s1)*in1, accum=sum(out).

    Follows the documented dve_ops authoring flow (append to OPS), done at
    runtime because the concourse tree is read-only here.
    """
    import concourse.dve_ops as DO
    from concourse.dve_spec import Spec, Src0, Src1, C0, C1, Zero, maxx, lower, _has_src1
    from concourse.dve_uop import DveOpSpec
    from operator import add as _add

    name = "ABS_AFFINE_MUL_REDUCE_ANT"
    for op in DO.OPS:
        if op.name == name:
            return op

    def ref(in0, in1, s0, s1, imm2):
        b = ((np.abs(in0.astype(np.float32)) * s0 + s1) * in1).astype(np.float32)
        return b, b.reshape(b.shape[0], -1).sum(axis=-1, keepdims=True)

    spec = Spec(
        body=(maxx(Src0, Zero - Src0) * C0 + C1) * Src1,
        accum=_add,
        accum_init=Zero,
        reference=ref,
    )
    probe = DO.DveOp(name, spec, subdim=False, uops_sha={})
    DO.OPS.append(probe)
    DO.CUSTOM_DVE_SPECS[name] = spec
    DO._SUB_OPCODE_FOR_NAME[name] = DO._CUSTOM_DVE_ROW_BASE + len(DO.OPS) - 1
    shas = {}
    for ver in ("v3", "v4"):
        r = DveOpSpec(
            name=name,
            opcode=DO.get_dve_sub_opcode(name),
            uops=lower(spec, ver=ver),
            rd1_en=_has_src1(spec),
        )
        shas[ver] = r.sha(ver)
    final = DO.DveOp(name, spec, subdim=False, uops_sha=shas)
    DO.OPS[-1] = final
    return final


def band_matrix_blocks() -> np.ndarray:
    """Three distinct band blocks [128, 3*128]: 0=diag |pi-po|<=15,
    1=below (pi-po>=113), 2=above (po-pi>=113); all scaled by 1/961."""
    import ml_dtypes

    pi = np.arange(128)[:, None]
    po = np.arange(128)[None, :]
    b0 = (np.abs(pi - po) <= 15).astype(np.float32)
    b1 = (pi - po >= 113).astype(np.float32)
    b2 = (po - pi >= 113).astype(np.float32)
    return np.ascontiguousarray(
        np.concatenate([b0, b1, b2], axis=1) * QSCALE
    ).astype(ml_dtypes.bfloat16)


def build_nc(n_img: int = IMG_PER_CORE) -> bacc.Bacc:
    register_custom_ops()
    nc = bacc.Bacc("TRN2", target_bir_lowering=False, debug=False)
    pred_d = nc.dram_tensor("pb", [n_img, H, W], BF16, kind="ExternalInput")
    targ_d = nc.dram_tensor("tb5", [n_img, H, W], BF16, kind="ExternalInput")
    bb_d = nc.dram_tensor("bband", [128, 3 * 128], BF16, kind="ExternalInput")
    negI_d = nc.dram_tensor("negident", [128, 128], BF16, kind="ExternalInput")
    ones_d = nc.dram_tensor("ones1", [128, 1], BF16, kind="ExternalInput")
    acc_d = nc.dram_tensor("acc", [128, 34], F32, kind="ExternalOutput")
    cs_d = nc.dram_tensor("csq", [4, 512], F32, kind="ExternalOutput")
    zc_d = nc.dram_tensor("zcp", [128, 2, 128], BF16, kind="ExternalOutput")

    with tile.TileContext(nc) as tc:
        _body(tc, pred_d, targ_d, bb_d, negI_d, ones_d, acc_d, cs_d, zc_d, n_img)
    nc.compile()
    return nc


def _body(tc, pred_d, targ_d, bb_d, negI_d, ones_d, acc_d, cs_d, zc_d, n_img):
    from concourse.dve_ops import TENSOR_TENSOR_REDUCE
    ABS_AMR = register_custom_ops()

    nc = tc.nc
    ACTF = mybir.ActivationFunctionType
    MULT = mybir.AluOpType.mult
    ADD = mybir.AluOpType.add
    SUB = mybir.AluOpType.subtract
    nhalf = 2 * n_img

    with (
        tc.tile_pool(name="const", bufs=1) as constp,
        tc.tile_pool(name="sc", bufs=1) as scp,
        tc.tile_pool(name="q", bufs=1) as qp,
        tc.tile_pool(name="u", bufs=1) as up,
        tc.tile_pool(name="qf", bufs=1) as qfp,
        tc.tile_pool(name="xd", bufs=1) as xdp,
        tc.tile_pool(name="dps", bufs=1, space=bass.MemorySpace.PSUM) as dpsp,
        tc.tile_pool(name="zps", bufs=1, space=bass.MemorySpace.PSUM) as zpsp,
        tc.tile_pool(name="csps", bufs=1, space=bass.MemorySpace.PSUM) as csp,
    ):
        bb = constp.tile([128, 3 * 128], BF16)
        negI = constp.tile([128, 128], BF16)
        ones1 = constp.tile([128, 1], BF16)
        acc = constp.tile([128, 34], F32)
        s1bufs = [constp.tile([128, RB, SROW], BF16, tag=f"s1_{k}", name=f"s1_{k}")
                  for k in range(4)]
        Pf_t = [constp.tile([128, 2048], BF16, tag=f"pf_{k}", name=f"pf_{k}")
                for k in range(n_img)]
        Fbig = constp.tile([128, nhalf, 1024], BF16)
        csev = constp.tile([128, 1024], F32)
        zcp = constp.tile([128, 2, 128], BF16)

        # input DMA triggers are emitted first (scalar HWDGE queue ~184GB/s)
        _dma_emitted = []

        # priming: custom-DVE uop table, gpsimd IRAM, sigmoid table
        zb = constp.tile([128, 1], F32)
        nc.vector.memset(zb[:], 0.0)
        pr0 = constp.tile([128, 1], F32)
        pr1 = constp.tile([128, 1], F32)
        nc.vector.affine_mul_reduce(pr0[:], pr1[:], zb[:], zb[:], 1.0, 0.0)
        prg = constp.tile([128, 1], F32)
        nc.gpsimd.tensor_tensor(prg[:], zb[:], zb[:], MULT)
        pra = constp.tile([128, 1], F32)
        nc.scalar.activation(pra[:], zb[:], ACTF.Sigmoid)

        for k in range(4):
            nc.vector.memset(s1bufs[k][:, :, 0:PADL], 0.0)
            nc.vector.memset(s1bufs[k][:, :, 544:SROW], 0.0)

        nc.sync.dma_start(bb[:], bb_d.ap())
        nc.sync.dma_start(negI[:], negI_d.ap())
        nc.sync.dma_start(ones1[:], ones_d.ap())

        cs_banks = [csp.tile([128, 512], F32, tag="cs0", name="cs0"),
                    csp.tile([128, 512], F32, tag="cs1", name="cs1")]
        zps = [zpsp.tile([128, 128], F32, tag="zA", name="zA"),
               zpsp.tile([128, 128], F32, tag="zB", name="zB")]

        sc_t, dps_t, q_t, u_t, qf_t = {}, {}, {}, {}, {}

        # distribute input DMAs across per-engine HW queues (4x bandwidth)
        s1_eng = [nc.scalar] * 4
        pf_eng = [nc.scalar] * 4

        def emit_dma(i):
            s1 = s1bufs[i % 4]
            s1_eng[i].dma_start(
                s1[:, :, PADL:544],
                targ_d.ap()[i].rearrange("(rb p) w -> p rb w", p=128),
            )
            pf_eng[i].dma_start(
                Pf_t[i][:].rearrange("p (rb w) -> p rb w", w=W),
                pred_d.ap()[i].rearrange("(rb p) w -> p rb w", p=128),
            )

        def emit_scan(i):
            s1 = s1bufs[i % 4]
            sc = scp.tile([128, RB, SROW], BF16, tag=f"sc_{i % 2}", name=f"sc_{i}")
            sc_t[i] = sc
            flat_in = s1[:].rearrange("p rb w -> p (rb w)")
            flat_out = sc[:].rearrange("p rb w -> p (rb w)")
            total = RB * SROW - (PADL + 1)
            nc.vector.tensor_tensor_scan(
                flat_out[:, 0:total],
                flat_in[:, PADL:PADL + total],
                flat_in[:, 1:1 + total],
                0.0, ADD, SUB,
            )

        def emit_hpool(i, h):
            s1 = s1bufs[i % 4]
            sc = sc_t[i]
            j = 2 * i + h
            rows = (2 * h, 2 * h + 1)
            dps = dpsp.tile([128, 2, 512], F32, tag=f"dps_{j % 2}", name=f"dps_{j}")
            dps_t[j] = dps
            for k, ro in enumerate(rows):
                ris = [r for r in (ro - 1, ro, ro + 1) if 0 <= r < RB]
                for m, ri in enumerate(ris):
                    blk = 0 if ri == ro else (1 if ri == ro - 1 else 2)
                    nc.tensor.matmul(
                        dps[:, k, :],
                        bb[:, blk * 128:(blk + 1) * 128],
                        sc[:, ri, 15:527],
                        start=(m == 0), stop=False,
                    )
                nc.tensor.matmul(
                    dps[:, k, :], negI[:], s1[:, ro, PADL:544],
                    start=False, stop=True,
                )

        def emit_trace(i):
            s1 = s1bufs[i % 4]
            Pf = Pf_t[i]
            for blk in range(16):
                rb, cb = divmod(blk, 4)
                gblk = 16 * i + blk
                nc.tensor.matmul(
                    zps[blk % 2][:],
                    Pf[:, blk * 128:(blk + 1) * 128],
                    s1[:, rb, PADL + cb * 128:PADL + (cb + 1) * 128],
                    start=(gblk < 2), stop=(gblk >= 16 * n_img - 2),
                )

        def emit_sigmoid(i, h):
            j = 2 * i + h
            nc.scalar.activation(Fbig[:, j, :], Pf_t[i][:, h * 1024:(h + 1) * 1024],
                                 ACTF.Sigmoid, scale=-1.0,
                                 accum_out=acc[:, 8 + j:9 + j])

        def emit_abs(i, h):
            j = 2 * i + h
            q = qp.tile([128, 2, 512], BF16, tag=f"q_{j % 2}", name=f"q_{j}")
            q_t[j] = q
            nc.scalar.activation(q[:], dps_t[j][:], ACTF.Abs,
                                 accum_out=acc[:, j:j + 1])

        def emit_u(i, h):
            j = 2 * i + h
            s1 = s1bufs[i % 4]
            rows = (2 * h, 2 * h + 1)
            t5v = s1[:, rows[0]:rows[0] + 2, PADL:544]
            u = up.tile([128, 2, 512], BF16, tag=f"u_{j % 2}", name=f"u_{j}")
            u_t[j] = u
            nc.vector._custom_dve(
                ABS_AMR, out=u[:], accum_out=acc[:, 16 + j:17 + j],
                in0=dps_t[j][:], in1=t5v, s0=1.0, s1=1.0,
            )

        def emit_qf(i, h):
            j = 2 * i + h
            qf = qfp.tile([128, 1024], BF16, tag=f"qf_{j}", name=f"qf_{j}")
            qf_t[j] = qf
            qflat = q_t[j][:].rearrange("p k w -> p (k w)")
            nc.gpsimd.tensor_tensor(qf[:], qflat, Fbig[:, j, :], MULT)

        def emit_x(i, h):
            j = 2 * i + h
            xd = xdp.tile([128, 1024], BF16, tag=f"xd_{j % 2}", name=f"xd_{j}")
            uflat = u_t[j][:].rearrange("p k w -> p (k w)")
            nc.vector._custom_dve(
                TENSOR_TENSOR_REDUCE, out=xd[:],
                accum_out=acc[:, 24 + j:25 + j],
                in0=uflat, in1=Fbig[:, j, :], s0=0.0, s1=1.0,
            )

        # ---- software-pipelined emission --------------------------------
        for k in range(4):
            emit_dma(k)
        emit_scan(0)
        emit_trace(0)          # PE filler while scan_0 runs
        emit_sigmoid(0, 0)
        emit_sigmoid(0, 1)
        for i in range(n_img):
            if i + 1 < n_img:
                emit_scan(i + 1)           # DVE: scan ahead
                emit_sigmoid(i + 1, 0)     # ACT: sigmoid ahead (DMA-dep only)
                emit_sigmoid(i + 1, 1)
            for h in range(2):
                emit_hpool(i, h)
            for h in range(2):
                emit_abs(i, h)
                emit_u(i, h)
                emit_qf(i, h)
                emit_x(i, h)
            if i + 1 < n_img:
                emit_trace(i + 1)          # PE filler while scan_{i+2} runs

        # ---- tail phase -------------------------------------------------
        # colsum groups (qF all alive)
        for i in range(n_img):
            bank = cs_banks[0] if i < 3 else cs_banks[1]
            bp = 32 * i if i < 3 else 0
            for h in range(2):
                qf = qf_t[2 * i + h]
                for b in range(2):
                    nc.tensor.matmul(
                        bank[bp:bp + 1, :], ones1[:], qf[:, b * 512:(b + 1) * 512],
                        start=(h == 0 and b == 0), stop=(h == 1 and b == 1),
                    )
        # Ln over F pairs (one natural_log table load)
        lnscr = constp.tile([128, 4096], BF16)
        for pr in range(n_img // 2):
            nc.scalar.activation(
                lnscr[:],
                Fbig[:, 4 * pr:4 * pr + 4, :].rearrange("p a b -> p (a b)"),
                ACTF.Ln, accum_out=acc[:, 32 + pr:33 + pr],
            )
        # evacs (Copy: filler in every set)
        for ch in range(2):
            nc.scalar.activation(zcp[:, ch, :], zps[ch][:], ACTF.Copy)
        nc.scalar.activation(csev[:, 0:512], cs_banks[0][:], ACTF.Copy)
        nc.scalar.activation(csev[:, 512:1024], cs_banks[1][:], ACTF.Copy)

        nc.sync.dma_start(acc_d.ap(), acc[:])
        for r, (p0, c0) in enumerate([(0, 0), (32, 0), (64, 0), (0, 512)]):
            nc.sync.dma_start(cs_d.ap()[r:r + 1, :], csev[p0:p0 + 1, c0:c0 + 512])
        nc.sync.dma_start(zc_d.ap(), zcp[:])


def combine(results, n_img_total):
    """results: list of dicts with acc [128,34], csq [128,1024], zcp [...]."""
    n_img = IMG_PER_CORE
    loss_terms = []
    g_total = 0.0
    for r in results:
        a = r["acc"].astype(np.float64)
        cs = r["csq"].astype(np.float64)
        zc = r["zcp"].astype(np.float64)   # [128, 2, 128] global chains
        sq_h = a[:, 0:8].sum(axis=0)       # per half
        sF_h = a[:, 8:16].sum(axis=0)
        su5_h = a[:, 16:24].sum(axis=0)
        sx5_h = a[:, 24:32].sum(axis=0)
        slnF = a[:, 32:34].sum()           # global over core
        g_total += -slnF
        szt5 = sum(np.trace(zc[:, ch, :]) for ch in range(2))
        g_total += -szt5 / 5.0
        for i in range(n_img):
            sq = sq_h[2 * i] + sq_h[2 * i + 1]
            sF = sF_h[2 * i] + sF_h[2 * i + 1]
            su5 = su5_h[2 * i] + su5_h[2 * i + 1]
            sx5 = sx5_h[2 * i] + sx5_h[2 * i + 1]
            row = cs[i]
            sqF = row.sum()
            A = NPIX + sq
            sv = sF + sqF
            B = (su5 - sx5) / 5.0
            C = A - sv + su5 / 5.0
            w_iou = 1.0 - (B + 1.0 + SMOOTH) / (C - B + 1.0 + SMOOTH)
            loss_terms.append((A, w_iou))
    bce = g_total / (n_img_total * NPIX)
    total = 0.0
    for A, w_iou in loss_terms:
        w_bce = (A * bce + SMOOTH) / (A + SMOOTH)
        total += w_bce + w_iou
    return np.float32(total / n_img_total)


def make_inputs(y_pred: np.ndarray, y_target: np.ndarray):
    import ml_dtypes

    pred = np.ascontiguousarray(np.asarray(y_pred, dtype=np.float32).reshape(-1, H, W))
    targ = np.ascontiguousarray(np.asarray(y_target, dtype=np.float32).reshape(-1, H, W))
    pb = pred.astype(ml_dtypes.bfloat16)
    tb5 = (5.0 * targ).astype(ml_dtypes.bfloat16)
    bb = band_matrix_blocks()
    negI = (-np.eye(128, dtype=np.float32)).astype(ml_dtypes.bfloat16)
    ones1 = np.ones((128, 1), dtype=np.float32).astype(ml_dtypes.bfloat16)
    in_maps = [
        {
            "pb": np.ascontiguousarray(pb[c * IMG_PER_CORE:(c + 1) * IMG_PER_CORE]),
            "tb5": np.ascontiguousarray(tb5[c * IMG_PER_CORE:(c + 1) * IMG_PER_CORE]),
            "bband": bb,
            "negident": negI,
            "ones1": ones1,
        }
        for c in range(N_CORES)
    ]
    return in_maps, pred.shape[0]


def kernel(y_pred: np.ndarray, y_target: np.ndarray) -> np.ndarray:
    in_maps, n_total = make_inputs(y_pred, y_target)
    nc = build_nc(IMG_PER_CORE)
    res = run_bass_kernel_spmd(nc, in_maps, list(range(N_CORES)))
    return np.asarray(combine([res.results[c] for c in range(N_CORES)], n_total))


# revision 3
# speedup vs baseline: 1.0095x; 1.0095x over previous
"""AdptWeightBCEDiceLoss Trainium2 kernel, v2 (w-pool-first redesign).

Full inputs y_pred/y_target [32,1,512,512] f32 -> scalar f32 loss.
8 cores x 4 images, data parallel.

Per-core pipeline (half-image units j=0..7, image i=j//2), with t5 = 5t:
  DMA:  s1_i = t5 padded rows [128,4,560] bf16 ([32:544) data), Pf_i bf16
  DVE:  fused 4-row scan -> sc_i (31-wide running w-boxsum, bf16)
  PE:   h-pool band matmul (bb = band/961) + (-I)*t5 accumulate
        -> D'_j = 5*avgpool - t5 in psum f32 [128,2,512]
  ACT:  q_j = |D'_j| -> SBUF bf16, accum -> sq_j  (abs, sigmoid-set filler)
  ACT:  F_j = sigmoid(-Pf_j), accum -> sF_j
  DVE:  ABS_AMR custom op: u_j = (|D'_j|+1)*t5, accum -> su5_j
  GpS:  qF_j = q_j * F_j
  PE:   colsum matmuls (ones stationary) over qF_j -> per-image psum row
  DVE:  TTR custom op: (u_j*F_j) accum -> sx5_j
  PE:   trace matmuls diag(Pf_i^T t5_i) -> zpsA/zpsB per image
  ACT:  phase 2: Ln(F) pairs, accum -> global sum ln F; psum evacs (Copy)
Host combines in float64:
  A = N + sq, sv = sF + colsum(qF), B = (su5-sx5)/5, C = A - sv + su5/5,
  bce = (-sum lnF - sum(x*t5)/5) / (32*N), w_iou = 1-(B+1+eps)/(C-B+1+eps),
  loss = mean(bce*~1 + w_iou)   [w_bce ~= bce; eps-correction negligible]
"""

import numpy as np

import concourse.bacc as bacc
import concourse.bass as bass
import concourse.tile as tile
from concourse import mybir
from concourse.bass_utils import run_bass_kernel_spmd

F32 = mybir.dt.float32
BF16 = mybir.dt.bfloat16
F8 = mybir.dt.float8e4

H = W = 512
RB = 4
KPOOL = 31
PADL = 32            # left pad (31 zeros needed; 32 keeps 4B alignment)
SROW = 560           # padded row: [0:32) zeros | [32:544) data | [544:560) zeros
NPIX = H * W
N_CORES = 8
IMG_PER_CORE = 4
NHALF = 2 * IMG_PER_CORE
SMOOTH = 1e-8
QSCALE = 1.0 / (KPOOL * KPOOL)


def register_custom_ops():
    """Register ABS_AFFINE_MUL_REDUCE: out=(|in0|*s0+s1)*in1, accum=sum(out).

    Follows the documented dve_ops authoring flow (append to OPS), done at
    runtime because the concourse tree is read-only here.
    """
    import concourse.dve_ops as DO
    from concourse.dve_spec import Spec, Src0, Src1, C0, C1, Zero, maxx, lower, _has_src1
    from concourse.dve_uop import DveOpSpec
    from operator import add as _add

    name = "ABS_AFFINE_MUL_REDUCE_ANT"
    for op in DO.OPS:
        if op.name == name:
            return op

    def ref(in0, in1, s0, s1, imm2):
        b = ((np.abs(in0.astype(np.float32)) * s0 + s1) * in1).astype(np.float32)
        return b, b.reshape(b.shape[0], -1).sum(axis=-1, keepdims=True)

    spec = Spec(
        body=(maxx(Src0, Zero - Src0) * C0 + C1) * Src1,
        accum=_add,
        accum_init=Zero,
        reference=ref,
    )
    probe = DO.DveOp(name, spec, subdim=False, uops_sha={})
    DO.OPS.append(probe)
    DO.CUSTOM_DVE_SPECS[name] = spec
    DO._SUB_OPCODE_FOR_NAME[name] = DO._CUSTOM_DVE_ROW_BASE + len(DO.OPS) - 1
    shas = {}
    for ver in ("v3", "v4"):
        r = DveOpSpec(
            name=name,
            opcode=DO.get_dve_sub_opcode(name),
            uops=lower(spec, ver=ver),
            rd1_en=_has_src1(spec),
        )
        shas[ver] = r.sha(ver)
    final = DO.DveOp(name, spec, subdim=False, uops_sha=shas)
    DO.OPS[-1] = final
    return final


def band_matrix_blocks() -> np.ndarray:
    """Three distinct band blocks [128, 3*128]: 0=diag |pi-po|<=15,
    1=below (pi-po>=113), 2=above (po-pi>=113); all scaled by 1/961."""
    import ml_dtypes

    pi = np.arange(128)[:, None]
    po = np.arange(128)[None, :]
    b0 = (np.abs(pi - po) <= 15).astype(np.float32)
    b1 = (pi - po >= 113).astype(np.float32)
    b2 = (po - pi >= 113).astype(np.float32)
    return np.ascontiguousarray(
        np.concatenate([b0, b1, b2], axis=1) * QSCALE
    ).astype(ml_dtypes.bfloat16)


def build_nc(n_img: int = IMG_PER_CORE) -> bacc.Bacc:
    register_custom_ops()
    nc = bacc.Bacc("TRN2", target_bir_lowering=False, debug=False)
    pred_d = nc.dram_tensor("pb", [n_img, H, W], F8, kind="ExternalInput")
    targ_d = nc.dram_tensor("tb5", [n_img, H, W], F8, kind="ExternalInput")
    bb_d = nc.dram_tensor("bband", [128, 3 * 128], BF16, kind="ExternalInput")
    negI_d = nc.dram_tensor("negident", [128, 128], F8, kind="ExternalInput")
    ones_d = nc.dram_tensor("ones1", [128, 1], BF16, kind="ExternalInput")
    acc_d = nc.dram_tensor("acc", [128, 34], F32, kind="ExternalOutput")
    cs_d = nc.dram_tensor("csq", [6, 512], F32, kind="ExternalOutput")
    zc_d = nc.dram_tensor("zcp", [128, 2, 128], BF16, kind="ExternalOutput")

    with tile.TileContext(nc) as tc:
        _body(tc, pred_d, targ_d, bb_d, negI_d, ones_d, acc_d, cs_d, zc_d, n_img)
    nc.compile()
    return nc


def _body(tc, pred_d, targ_d, bb_d, negI_d, ones_d, acc_d, cs_d, zc_d, n_img):
    from concourse.dve_ops import TENSOR_TENSOR_REDUCE
    ABS_AMR = register_custom_ops()

    nc = tc.nc
    ACTF = mybir.ActivationFunctionType
    MULT = mybir.AluOpType.mult
    ADD = mybir.AluOpType.add
    SUB = mybir.AluOpType.subtract
    nhalf = 2 * n_img

    with (
        tc.tile_pool(name="const", bufs=1) as constp,
        tc.tile_pool(name="sc", bufs=1) as scp,
        tc.tile_pool(name="q", bufs=1) as qp,
        tc.tile_pool(name="u", bufs=1) as up,
        tc.tile_pool(name="qf", bufs=1) as qfp,
        tc.tile_pool(name="xd", bufs=1) as xdp,
        tc.tile_pool(name="dps", bufs=1, space=bass.MemorySpace.PSUM) as dpsp,
        tc.tile_pool(name="zps", bufs=1, space=bass.MemorySpace.PSUM) as zpsp,
        tc.tile_pool(name="csps", bufs=1, space=bass.MemorySpace.PSUM) as csp,
    ):
        bb = constp.tile([128, 3 * 128], BF16)
        negI = constp.tile([128, 128], F8)
        ones1 = constp.tile([128, 1], BF16)
        acc = constp.tile([128, 34], F32)
        s1bufs = [constp.tile([128, RB, SROW], F8, tag=f"s1_{k}", name=f"s1_{k}")
                  for k in range(4)]
        Pf_t = [constp.tile([128, 2048], F8, tag=f"pf_{k}", name=f"pf_{k}")
                for k in range(n_img)]
        Fbig = constp.tile([128, nhalf, 1024], BF16)
        csev = constp.tile([128, 1024], F32)
        zcp = constp.tile([128, 2, 128], BF16)

        # input DMA triggers FIRST: they must be the scalar queue's first
        # instructions so the HWDGE spin-up overlaps kernel boilerplate
        for _i in range(n_img):
            nc.scalar.dma_start(
                s1bufs[_i % 4][:, :, PADL:544],
                targ_d.ap()[_i].rearrange("(rb p) w -> p rb w", p=128),
            )
            nc.scalar.dma_start(
                Pf_t[_i][:].rearrange("p (rb w) -> p rb w", w=W),
                pred_d.ap()[_i].rearrange("(rb p) w -> p rb w", p=128),
            )

        # priming: custom-DVE uop table, gpsimd IRAM, sigmoid table
        zb = constp.tile([128, 1], F32)
        nc.vector.memset(zb[:], 0.0)
        pr0 = constp.tile([128, 1], F32)
        pr1 = constp.tile([128, 1], F32)
        nc.vector.affine_mul_reduce(pr0[:], pr1[:], zb[:], zb[:], 1.0, 0.0)
        prg = constp.tile([128, 1], F32)
        nc.gpsimd.tensor_tensor(prg[:], zb[:], zb[:], MULT)
        pra = constp.tile([128, 1], F32)
        nc.scalar.activation(pra[:], zb[:], ACTF.Sigmoid)

        for k in range(4):
            nc.vector.memset(s1bufs[k][:, :, 0:PADL], 0.0)
            nc.vector.memset(s1bufs[k][:, :, 544:SROW], 0.0)

        nc.sync.dma_start(bb[:], bb_d.ap())
        nc.sync.dma_start(negI[:], negI_d.ap())
        nc.sync.dma_start(ones1[:], ones_d.ap())

        cs_banks = [csp.tile([128, 512], F32, tag="cs0", name="cs0"),
                    csp.tile([128, 512], F32, tag="cs1", name="cs1")]
        zps = [zpsp.tile([128, 128], F32, tag="zA", name="zA"),
               zpsp.tile([128, 128], F32, tag="zB", name="zB")]

        sc_t, dps_t, q_t, u_t, qf_t = {}, {}, {}, {}, {}

        # distribute input DMAs across per-engine HW queues (4x bandwidth)
        s1_eng = [nc.scalar] * 4
        pf_eng = [nc.scalar] * 4

        def emit_dma(i):
            pass

        def emit_scan(i):
            s1 = s1bufs[i % 4]
            sc = scp.tile([128, RB, SROW], BF16, tag=f"sc_{i % 2}", name=f"sc_{i}")
            sc_t[i] = sc
            flat_in = s1[:].rearrange("p rb w -> p (rb w)")
            flat_out = sc[:].rearrange("p rb w -> p (rb w)")
            total = RB * SROW - (PADL + 1)
            nc.vector.tensor_tensor_scan(
                flat_out[:, 0:total],
                flat_in[:, PADL:PADL + total],
                flat_in[:, 1:1 + total],
                0.0, ADD, SUB,
            )

        def emit_hpool(i, h):
            s1 = s1bufs[i % 4]
            sc = sc_t[i]
            j = 2 * i + h
            rows = (2 * h, 2 * h + 1)
            dps = dpsp.tile([128, 2, 512], F32, tag=f"dps_{j % 2}", name=f"dps_{j}")
            dps_t[j] = dps
            for k, ro in enumerate(rows):
                ris = [r for r in (ro - 1, ro, ro + 1) if 0 <= r < RB]
                for m, ri in enumerate(ris):
                    blk = 0 if ri == ro else (1 if ri == ro - 1 else 2)
                    nc.tensor.matmul(
                        dps[:, k, :],
                        bb[:, blk * 128:(blk + 1) * 128],
                        sc[:, ri, 15:527],
                        start=(m == 0), stop=False,
                    )
                nc.tensor.matmul(
                    dps[:, k, :], negI[:], s1[:, ro, PADL:544],
                    start=False, stop=True,
                )

        def emit_trace(i):
            s1 = s1bufs[i % 4]
            Pf = Pf_t[i]
            for blk in range(16):
                rb, cb = divmod(blk, 4)
                gblk = 16 * i + blk
                nc.tensor.matmul(
                    zps[blk % 2][:],
                    Pf[:, blk * 128:(blk + 1) * 128],
                    s1[:, rb, PADL + cb * 128:PADL + (cb + 1) * 128],
                    start=(gblk < 2), stop=(gblk >= 16 * n_img - 2),
                )

        def emit_sigmoid(i, h):
            j = 2 * i + h
            nc.scalar.activation(Fbig[:, j, :], Pf_t[i][:, h * 1024:(h + 1) * 1024],
                                 ACTF.Sigmoid, scale=-1.0,
                                 accum_out=acc[:, 8 + j:9 + j])

        def emit_abs(i, h):
            j = 2 * i + h
            q = qp.tile([128, 2, 512], BF16, tag=f"q_{j % 2}", name=f"q_{j}")
            q_t[j] = q
            nc.scalar.activation(q[:], dps_t[j][:], ACTF.Abs,
                                 accum_out=acc[:, j:j + 1])

        def emit_u(i, h):
            j = 2 * i + h
            s1 = s1bufs[i % 4]
            rows = (2 * h, 2 * h + 1)
            t5v = s1[:, rows[0]:rows[0] + 2, PADL:544]
            u = up.tile([128, 2, 512], BF16, tag=f"u_{j % 2}", name=f"u_{j}")
            u_t[j] = u
            nc.vector._custom_dve(
                ABS_AMR, out=u[:], accum_out=acc[:, 16 + j:17 + j],
                in0=dps_t[j][:], in1=t5v, s0=1.0, s1=1.0,
            )

        def emit_qf(i, h):
            j = 2 * i + h
            qf = qfp.tile([128, 1024], BF16, tag=f"qf_{j}", name=f"qf_{j}")
            qf_t[j] = qf
            qflat = q_t[j][:].rearrange("p k w -> p (k w)")
            nc.gpsimd.tensor_tensor(qf[:], qflat, Fbig[:, j, :], MULT)

        def emit_x(i, h):
            j = 2 * i + h
            uflat = u_t[j][:].rearrange("p k w -> p (k w)")
            xd = xdp.tile([128, 1024], BF16, tag=f"xd_{j % 2}", name=f"xd_{j}")
            nc.vector._custom_dve(
                TENSOR_TENSOR_REDUCE, out=xd[:],
                accum_out=acc[:, 24 + j:25 + j],
                in0=uflat, in1=Fbig[:, j, :], s0=0.0, s1=1.0,
            )

        # ---- software-pipelined emission --------------------------------
        emit_scan(0)
        emit_trace(0)          # PE filler while scan_0 runs
        emit_sigmoid(0, 0)
        emit_sigmoid(0, 1)
        for i in range(n_img):
            if i + 1 < n_img:
                emit_scan(i + 1)           # DVE: scan ahead
                emit_sigmoid(i + 1, 0)     # ACT: sigmoid ahead (DMA-dep only)
                emit_sigmoid(i + 1, 1)
            for h in range(2):
                emit_hpool(i, h)
            for h in range(2):
                emit_abs(i, h)
                emit_u(i, h)
                emit_qf(i, h)
                emit_x(i, h)
            if i + 1 < n_img:
                emit_trace(i + 1)          # PE filler while scan_{i+2} runs

        # ---- tail phase -------------------------------------------------
        # colsum groups (qF all alive)
        for i in range(n_img):
            bank = cs_banks[0] if i < 3 else cs_banks[1]
            bp = 32 * i if i < 3 else 0
            for h in range(2):
                qf = qf_t[2 * i + h]
                for b in range(2):
                    nc.tensor.matmul(
                        bank[bp:bp + 1, :], ones1[:], qf[:, b * 512:(b + 1) * 512],
                        start=(h == 0 and b == 0), stop=(h == 1 and b == 1),
                    )
        # Ln over F pairs (one natural_log table load)
        lnscr = constp.tile([128, 4096], BF16)
        for pr in range(n_img // 2):
            nc.scalar.activation(
                lnscr[:],
                Fbig[:, 4 * pr:4 * pr + 4, :].rearrange("p a b -> p (a b)"),
                ACTF.Ln, accum_out=acc[:, 32 + pr:33 + pr],
            )
        # evacs (Copy: filler in every set)
        for ch in range(2):
            nc.scalar.activation(zcp[:, ch, :], zps[ch][:], ACTF.Copy)
        nc.scalar.activation(csev[:, 0:512], cs_banks[0][:], ACTF.Copy)
        nc.scalar.activation(csev[:, 512:1024], cs_banks[1][:], ACTF.Copy)

        nc.scalar.dma_start(acc_d.ap(), acc[:])
        for r, (p0, c0) in enumerate([(0, 0), (32, 0), (64, 0), (0, 512),
                                      (32, 512), (64, 512)]):
            nc.scalar.dma_start(cs_d.ap()[r:r + 1, :], csev[p0:p0 + 1, c0:c0 + 512])
        nc.scalar.dma_start(zc_d.ap(), zcp[:])


def combine(results, n_img_total):
    """results: list of dicts with acc [128,34], csq [128,1024], zcp [...]."""
    n_img = IMG_PER_CORE
    loss_terms = []
    g_total = 0.0
    for r in results:
        a = r["acc"].astype(np.float64)
        cs = r["csq"].astype(np.float64)
        zc = r["zcp"].astype(np.float64)   # [128, 2, 128] global chains
        sq_h = a[:, 0:8].sum(axis=0)       # per half
        sF_h = a[:, 8:16].sum(axis=0)
        su5_h = a[:, 16:24].sum(axis=0)
        sx5_h = a[:, 24:32].sum(axis=0)
        slnF = a[:, 32:34].sum()           # global over core
        g_total += -slnF
        szt5 = sum(np.trace(zc[:, ch, :]) for ch in range(2))
        g_total += -szt5 / 5.0
        for i in range(n_img):
            sq = sq_h[2 * i] + sq_h[2 * i + 1]
            sF = sF_h[2 * i] + sF_h[2 * i + 1]
            su5 = su5_h[2 * i] + su5_h[2 * i + 1]
            sx5 = sx5_h[2 * i] + sx5_h[2 * i + 1]
            row = cs[i]
            sqF = row.sum()
            A = NPIX + sq
            sv = sF + sqF
            B = (su5 - sx5) / 5.0
            C = A - sv + su5 / 5.0
            w_iou = 1.0 - (B + 1.0 + SMOOTH) / (C - B + 1.0 + SMOOTH)
            loss_terms.append((A, w_iou))
    bce = g_total / (n_img_total * NPIX)
    total = 0.0
    for A, w_iou in loss_terms:
        w_bce = (A * bce + SMOOTH) / (A + SMOOTH)
        total += w_bce + w_iou
    return np.float32(total / n_img_total)


def make_inputs(y_pred: np.ndarray, y_target: np.ndarray):
    import ml_dtypes

    pred = np.ascontiguousarray(np.asarray(y_pred, dtype=np.float32).reshape(-1, H, W))
    targ = np.ascontiguousarray(np.asarray(y_target, dtype=np.float32).reshape(-1, H, W))
    pb = pred.astype(ml_dtypes.float8_e4m3)
    tb5 = (5.0 * targ).astype(ml_dtypes.float8_e4m3)
    bb = band_matrix_blocks()
    negI = (-np.eye(128, dtype=np.float32)).astype(ml_dtypes.float8_e4m3)
    ones1 = np.ones((128, 1), dtype=np.float32).astype(ml_dtypes.bfloat16)
    in_maps = [
        {
            "pb": np.ascontiguousarray(pb[c * IMG_PER_CORE:(c + 1) * IMG_PER_CORE]),
            "tb5": np.ascontiguousarray(tb5[c * IMG_PER_CORE:(c + 1) * IMG_PER_CORE]),
            "bband": bb,
            "negident": negI,
            "ones1": ones1,
        }
        for c in range(N_CORES)
    ]
    return in_maps, pred.shape[0]


def kernel(y_pred: np.ndarray, y_target: np.ndarray) -> np.ndarray:
    in_maps, n_total = make_inputs(y_pred, y_target)
    nc = build_nc(IMG_PER_CORE)
    res = run_bass_kernel_spmd(nc, in_maps, list(range(N_CORES)))
    return np.asarray(combine([res.results[c] for c in range(N_CORES)], n_total))


# revision 4
# speedup vs baseline: 1.0448x; 1.0349x over previous
"""AdptWeightBCEDiceLoss Trainium2 kernel, v2 (w-pool-first redesign).

Full inputs y_pred/y_target [32,1,512,512] f32 -> scalar f32 loss.
8 cores x 4 images, data parallel.

Per-core pipeline (half-image units j=0..7, image i=j//2), with t5 = 5t:
  DMA:  s1_i = t5 padded rows [128,4,560] bf16 ([32:544) data), Pf_i bf16
  DVE:  fused 4-row scan -> sc_i (31-wide running w-boxsum, bf16)
  PE:   h-pool band matmul (bb = band/961) + (-I)*t5 accumulate
        -> D'_j = 5*avgpool - t5 in psum f32 [128,2,512]
  ACT:  q_j = |D'_j| -> SBUF bf16, accum -> sq_j  (abs, sigmoid-set filler)
  ACT:  F_j = sigmoid(-Pf_j), accum -> sF_j
  DVE:  ABS_AMR custom op: u_j = (|D'_j|+1)*t5, accum -> su5_j
  GpS:  qF_j = q_j * F_j
  PE:   colsum matmuls (ones stationary) over qF_j -> per-image psum row
  DVE:  TTR custom op: (u_j*F_j) accum -> sx5_j
  PE:   trace matmuls diag(Pf_i^T t5_i) -> zpsA/zpsB per image
  ACT:  phase 2: Ln(F) pairs, accum -> global sum ln F; psum evacs (Copy)
Host combines in float64:
  A = N + sq, sv = sF + colsum(qF), B = (su5-sx5)/5, C = A - sv + su5/5,
  bce = (-sum lnF - sum(x*t5)/5) / (32*N), w_iou = 1-(B+1+eps)/(C-B+1+eps),
  loss = mean(bce*~1 + w_iou)   [w_bce ~= bce; eps-correction negligible]
"""

import numpy as np

import concourse.bacc as bacc
import concourse.bass as bass
import concourse.tile as tile
from concourse import mybir
from concourse.bass_utils import run_bass_kernel_spmd

F32 = mybir.dt.float32
BF16 = mybir.dt.bfloat16
F8 = mybir.dt.float8e4

H = W = 512
RB = 4
KPOOL = 31
PADL = 32            # left pad (31 zeros needed; 32 keeps 4B alignment)
SROW = 560           # padded row: [0:32) zeros | [32:544) data | [544:560) zeros
NPIX = H * W
N_CORES = 8
IMG_PER_CORE = 4
NHALF = 2 * IMG_PER_CORE
SMOOTH = 1e-8
QSCALE = 1.0 / (KPOOL * KPOOL)


def register_custom_ops():
    """Register ABS_AFFINE_MUL_REDUCE: out=(|in0|*s0+s1)*in1, accum=sum(out).

    Follows the documented dve_ops authoring flow (append to OPS), done at
    runtime because the concourse tree is read-only here.
    """
    import concourse.dve_ops as DO
    from concourse.dve_spec import Spec, Src0, Src1, C0, C1, Zero, maxx, lower, _has_src1
    from concourse.dve_uop import DveOpSpec
    from operator import add as _add

    name = "ABS_AFFINE_MUL_REDUCE_ANT"
    for op in DO.OPS:
        if op.name == name:
            return op

    def ref(in0, in1, s0, s1, imm2):
        b = ((np.abs(in0.astype(np.float32)) * s0 + s1) * in1).astype(np.float32)
        return b, b.reshape(b.shape[0], -1).sum(axis=-1, keepdims=True)

    spec = Spec(
        body=(maxx(Src0, Zero - Src0) * C0 + C1) * Src1,
        accum=_add,
        accum_init=Zero,
        reference=ref,
    )
    probe = DO.DveOp(name, spec, subdim=False, uops_sha={})
    DO.OPS.append(probe)
    DO.CUSTOM_DVE_SPECS[name] = spec
    DO._SUB_OPCODE_FOR_NAME[name] = DO._CUSTOM_DVE_ROW_BASE + len(DO.OPS) - 1
    shas = {}
    for ver in ("v3", "v4"):
        r = DveOpSpec(
            name=name,
            opcode=DO.get_dve_sub_opcode(name),
            uops=lower(spec, ver=ver),
            rd1_en=_has_src1(spec),
        )
        shas[ver] = r.sha(ver)
    final = DO.DveOp(name, spec, subdim=False, uops_sha=shas)
    DO.OPS[-1] = final
    return final


def band_matrix_blocks() -> np.ndarray:
    """Three distinct band blocks [128, 3*128]: 0=diag |pi-po|<=15,
    1=below (pi-po>=113), 2=above (po-pi>=113); all scaled by 1/961."""
    import ml_dtypes

    pi = np.arange(128)[:, None]
    po = np.arange(128)[None, :]
    b0 = (np.abs(pi - po) <= 15).astype(np.float32)
    b1 = (pi - po >= 113).astype(np.float32)
    b2 = (po - pi >= 113).astype(np.float32)
    return np.ascontiguousarray(
        np.concatenate([b0, b1, b2], axis=1) * QSCALE
    ).astype(ml_dtypes.bfloat16)


def build_nc(n_img: int = IMG_PER_CORE) -> bacc.Bacc:
    register_custom_ops()
    nc = bacc.Bacc("TRN2", target_bir_lowering=False, debug=False)
    pred_d = nc.dram_tensor("pb", [n_img, H, W], BF16, kind="ExternalInput")
    targ_d = nc.dram_tensor("tb5", [n_img, H, W], BF16, kind="ExternalInput")
    bb_d = nc.dram_tensor("bband", [128, 3 * 128], BF16, kind="ExternalInput")
    negI_d = nc.dram_tensor("negident", [128, 128], BF16, kind="ExternalInput")
    ones_d = nc.dram_tensor("ones1", [128, 1], BF16, kind="ExternalInput")
    acc_d = nc.dram_tensor("acc", [128, 34], F32, kind="ExternalOutput")
    cs_d = nc.dram_tensor("csq", [6, 512], F32, kind="ExternalOutput")
    zc_d = nc.dram_tensor("zcp", [128, 2, 128], BF16, kind="ExternalOutput")

    with tile.TileContext(nc) as tc:
        _body(tc, pred_d, targ_d, bb_d, negI_d, ones_d, acc_d, cs_d, zc_d, n_img)
    nc.compile()
    return nc


def _body(tc, pred_d, targ_d, bb_d, negI_d, ones_d, acc_d, cs_d, zc_d, n_img):
    from concourse.dve_ops import TENSOR_TENSOR_REDUCE
    ABS_AMR = register_custom_ops()

    nc = tc.nc
    ACTF = mybir.ActivationFunctionType
    MULT = mybir.AluOpType.mult
    ADD = mybir.AluOpType.add
    SUB = mybir.AluOpType.subtract
    nhalf = 2 * n_img

    with (
        tc.tile_pool(name="const", bufs=1) as constp,
        tc.tile_pool(name="sc", bufs=1) as scp,
        tc.tile_pool(name="q", bufs=1) as qp,
        tc.tile_pool(name="u", bufs=1) as up,
        tc.tile_pool(name="qf", bufs=1) as qfp,
        tc.tile_pool(name="xd", bufs=1) as xdp,
        tc.tile_pool(name="dps", bufs=1, space=bass.MemorySpace.PSUM) as dpsp,
        tc.tile_pool(name="zps", bufs=1, space=bass.MemorySpace.PSUM) as zpsp,
        tc.tile_pool(name="csps", bufs=1, space=bass.MemorySpace.PSUM) as csp,
    ):
        bb = constp.tile([128, 3 * 128], BF16)
        negI = constp.tile([128, 128], BF16)
        ones1 = constp.tile([128, 1], BF16)
        acc = constp.tile([128, 34], F32)
        s1bufs = [constp.tile([128, RB, SROW], BF16, tag=f"s1_{k}", name=f"s1_{k}")
                  for k in range(4)]
        Pf_t = [constp.tile([128, 2048], BF16, tag=f"pf_{k}", name=f"pf_{k}")
                for k in range(n_img)]
        Fbig = constp.tile([128, nhalf, 1024], BF16)
        csev = constp.tile([128, 1024], F32)
        zcp = constp.tile([128, 2, 128], BF16)

        # input DMA triggers FIRST: they must be the scalar queue's first
        # instructions so the HWDGE spin-up overlaps kernel boilerplate
        for _i in range(n_img):
            nc.scalar.dma_start(
                s1bufs[_i % 4][:, :, PADL:544],
                targ_d.ap()[_i].rearrange("(rb p) w -> p rb w", p=128),
            )
            nc.scalar.dma_start(
                Pf_t[_i][:].rearrange("p (rb w) -> p rb w", w=W),
                pred_d.ap()[_i].rearrange("(rb p) w -> p rb w", p=128),
            )

        # priming: custom-DVE uop table, gpsimd IRAM, sigmoid table
        zb = constp.tile([128, 1], F32)
        nc.vector.memset(zb[:], 0.0)
        pr0 = constp.tile([128, 1], F32)
        pr1 = constp.tile([128, 1], F32)
        nc.vector.affine_mul_reduce(pr0[:], pr1[:], zb[:], zb[:], 1.0, 0.0)
        prg = constp.tile([128, 1], F32)
        nc.gpsimd.tensor_tensor(prg[:], zb[:], zb[:], MULT)
        pra = constp.tile([128, 1], F32)
        nc.scalar.activation(pra[:], zb[:], ACTF.Sigmoid)

        for k in range(4):
            nc.vector.memset(s1bufs[k][:, :, 0:PADL], 0.0)
            nc.vector.memset(s1bufs[k][:, :, 544:SROW], 0.0)

        nc.sync.dma_start(bb[:], bb_d.ap())
        nc.sync.dma_start(negI[:], negI_d.ap())
        nc.sync.dma_start(ones1[:], ones_d.ap())

        cs_banks = [csp.tile([128, 512], F32, tag="cs0", name="cs0"),
                    csp.tile([128, 512], F32, tag="cs1", name="cs1")]
        zps = [zpsp.tile([128, 128], F32, tag="zA", name="zA"),
               zpsp.tile([128, 128], F32, tag="zB", name="zB")]

        sc_t, dps_t, q_t, u_t, qf_t = {}, {}, {}, {}, {}

        # distribute input DMAs across per-engine HW queues (4x bandwidth)
        s1_eng = [nc.scalar] * 4
        pf_eng = [nc.scalar] * 4

        def emit_dma(i):
            pass

        def emit_scan(i):
            s1 = s1bufs[i % 4]
            sc = scp.tile([128, RB, SROW], BF16, tag=f"sc_{i % 2}", name=f"sc_{i}")
            sc_t[i] = sc
            flat_in = s1[:].rearrange("p rb w -> p (rb w)")
            flat_out = sc[:].rearrange("p rb w -> p (rb w)")
            total = RB * SROW - (PADL + 1)
            nc.vector.tensor_tensor_scan(
                flat_out[:, 0:total],
                flat_in[:, PADL:PADL + total],
                flat_in[:, 1:1 + total],
                0.0, ADD, SUB,
            )

        def emit_hpool(i, h):
            s1 = s1bufs[i % 4]
            sc = sc_t[i]
            j = 2 * i + h
            rows = (2 * h, 2 * h + 1)
            dps = dpsp.tile([128, 2, 512], F32, tag=f"dps_{j % 2}", name=f"dps_{j}")
            dps_t[j] = dps
            for k, ro in enumerate(rows):
                ris = [r for r in (ro - 1, ro, ro + 1) if 0 <= r < RB]
                for m, ri in enumerate(ris):
                    blk = 0 if ri == ro else (1 if ri == ro - 1 else 2)
                    nc.tensor.matmul(
                        dps[:, k, :],
                        bb[:, blk * 128:(blk + 1) * 128],
                        sc[:, ri, 15:527],
                        start=(m == 0), stop=False,
                    )
                nc.tensor.matmul(
                    dps[:, k, :], negI[:], s1[:, ro, PADL:544],
                    start=False, stop=True,
                )

        def emit_trace(i):
            s1 = s1bufs[i % 4]
            Pf = Pf_t[i]
            for blk in range(16):
                rb, cb = divmod(blk, 4)
                gblk = 16 * i + blk
                nc.tensor.matmul(
                    zps[blk % 2][:],
                    Pf[:, blk * 128:(blk + 1) * 128],
                    s1[:, rb, PADL + cb * 128:PADL + (cb + 1) * 128],
                    start=(gblk < 2), stop=(gblk >= 16 * n_img - 2),
                )

        def emit_sigmoid(i, h):
            j = 2 * i + h
            nc.scalar.activation(Fbig[:, j, :], Pf_t[i][:, h * 1024:(h + 1) * 1024],
                                 ACTF.Sigmoid, scale=-1.0,
                                 accum_out=acc[:, 8 + j:9 + j])

        def emit_abs(i, h):
            j = 2 * i + h
            q = qp.tile([128, 2, 512], BF16, tag=f"q_{j % 2}", name=f"q_{j}")
            q_t[j] = q
            nc.scalar.activation(q[:], dps_t[j][:], ACTF.Abs,
                                 accum_out=acc[:, j:j + 1])

        def emit_u(i, h):
            j = 2 * i + h
            s1 = s1bufs[i % 4]
            rows = (2 * h, 2 * h + 1)
            t5v = s1[:, rows[0]:rows[0] + 2, PADL:544]
            u = up.tile([128, 2, 512], BF16, tag=f"u_{j % 2}", name=f"u_{j}")
            u_t[j] = u
            nc.vector._custom_dve(
                ABS_AMR, out=u[:], accum_out=acc[:, 16 + j:17 + j],
                in0=dps_t[j][:], in1=t5v, s0=1.0, s1=1.0,
            )

        def emit_qf(i, h):
            j = 2 * i + h
            qf = qfp.tile([128, 1024], BF16, tag=f"qf_{j}", name=f"qf_{j}")
            qf_t[j] = qf
            qflat = q_t[j][:].rearrange("p k w -> p (k w)")
            nc.gpsimd.tensor_tensor(qf[:], qflat, Fbig[:, j, :], MULT)

        def emit_x(i, h):
            j = 2 * i + h
            uflat = u_t[j][:].rearrange("p k w -> p (k w)")
            xd = xdp.tile([128, 1024], BF16, tag=f"xd_{j % 2}", name=f"xd_{j}")
            nc.vector._custom_dve(
                TENSOR_TENSOR_REDUCE, out=xd[:],
                accum_out=acc[:, 24 + j:25 + j],
                in0=uflat, in1=Fbig[:, j, :], s0=0.0, s1=1.0,
            )

        # ---- software-pipelined emission --------------------------------
        emit_scan(0)
        emit_trace(0)          # PE filler while scan_0 runs
        emit_sigmoid(0, 0)
        emit_sigmoid(0, 1)
        for i in range(n_img):
            if i + 1 < n_img:
                emit_scan(i + 1)           # DVE: scan ahead
                emit_sigmoid(i + 1, 0)     # ACT: sigmoid ahead (DMA-dep only)
                emit_sigmoid(i + 1, 1)
            for h in range(2):
                emit_hpool(i, h)
            for h in range(2):
                emit_abs(i, h)
                emit_u(i, h)
                emit_qf(i, h)
                emit_x(i, h)
            if i + 1 < n_img:
                emit_trace(i + 1)          # PE filler while scan_{i+2} runs

        # ---- tail phase -------------------------------------------------
        # colsum groups (qF all alive)
        for i in range(n_img):
            bank = cs_banks[0] if i < 3 else cs_banks[1]
            bp = 32 * i if i < 3 else 0
            for h in range(2):
                qf = qf_t[2 * i + h]
                for b in range(2):
                    nc.tensor.matmul(
                        bank[bp:bp + 1, :], ones1[:], qf[:, b * 512:(b + 1) * 512],
                        start=(h == 0 and b == 0), stop=(h == 1 and b == 1),
                    )
        # Ln over F pairs (one natural_log table load)
        lnscr = constp.tile([128, 4096], BF16)
        for pr in range(n_img // 2):
            nc.scalar.activation(
                lnscr[:],
                Fbig[:, 4 * pr:4 * pr + 4, :].rearrange("p a b -> p (a b)"),
                ACTF.Ln, accum_out=acc[:, 32 + pr:33 + pr],
            )
        # evacs (Copy: filler in every set)
        for ch in range(2):
            nc.scalar.activation(zcp[:, ch, :], zps[ch][:], ACTF.Copy)
        nc.scalar.activation(csev[:, 0:512], cs_banks[0][:], ACTF.Copy)
        nc.scalar.activation(csev[:, 512:1024], cs_banks[1][:], ACTF.Copy)

        nc.scalar.dma_start(acc_d.ap(), acc[:])
        for r, (p0, c0) in enumerate([(0, 0), (32, 0), (64, 0), (0, 512),
                                      (32, 512), (64, 512)]):
            nc.scalar.dma_start(cs_d.ap()[r:r + 1, :], csev[p0:p0 + 1, c0:c0 + 512])
        nc.scalar.dma_start(zc_d.ap(), zcp[:])


def combine(results, n_img_total):
    """results: list of dicts with acc [128,34], csq [128,1024], zcp [...]."""
    n_img = IMG_PER_CORE
    loss_terms = []
    g_total = 0.0
    for r in results:
        a = r["acc"].astype(np.float64)
        cs = r["csq"].astype(np.float64)
        zc = r["zcp"].astype(np.float64)   # [128, 2, 128] global chains
        sq_h = a[:, 0:8].sum(axis=0)       # per half
        sF_h = a[:, 8:16].sum(axis=0)
        su5_h = a[:, 16:24].sum(axis=0)
        sx5_h = a[:, 24:32].sum(axis=0)
        slnF = a[:, 32:34].sum()           # global over core
        g_total += -slnF
        szt5 = sum(np.trace(zc[:, ch, :]) for ch in range(2))
        g_total += -szt5 / 5.0
        for i in range(n_img):
            sq = sq_h[2 * i] + sq_h[2 * i + 1]
            sF = sF_h[2 * i] + sF_h[2 * i + 1]
            su5 = su5_h[2 * i] + su5_h[2 * i + 1]
            sx5 = sx5_h[2 * i] + sx5_h[2 * i + 1]
            row = cs[i]
            sqF = row.sum()
            A = NPIX + sq
            sv = sF + sqF
            B = (su5 - sx5) / 5.0
            C = A - sv + su5 / 5.0
            w_iou = 1.0 - (B + 1.0 + SMOOTH) / (C - B + 1.0 + SMOOTH)
            loss_terms.append((A, w_iou))
    bce = g_total / (n_img_total * NPIX)
    total = 0.0
    for A, w_iou in loss_terms:
        w_bce = (A * bce + SMOOTH) / (A + SMOOTH)
        total += w_bce + w_iou
    return np.float32(total / n_img_total)


def make_inputs(y_pred: np.ndarray, y_target: np.ndarray):
    import ml_dtypes

    pred = np.ascontiguousarray(np.asarray(y_pred, dtype=np.float32).reshape(-1, H, W))
    targ = np.ascontiguousarray(np.asarray(y_target, dtype=np.float32).reshape(-1, H, W))
    pb = pred.astype(ml_dtypes.bfloat16)
    tb5 = (5.0 * targ).astype(ml_dtypes.bfloat16)
    bb = band_matrix_blocks()
    negI = (-np.eye(128, dtype=np.float32)).astype(ml_dtypes.bfloat16)
    ones1 = np.ones((128, 1), dtype=np.float32).astype(ml_dtypes.bfloat16)
    in_maps = [
        {
            "pb": np.ascontiguousarray(pb[c * IMG_PER_CORE:(c + 1) * IMG_PER_CORE]),
            "tb5": np.ascontiguousarray(tb5[c * IMG_PER_CORE:(c + 1) * IMG_PER_CORE]),
            "bband": bb,
            "negident": negI,
            "ones1": ones1,
        }
        for c in range(N_CORES)
    ]
    return in_maps, pred.shape[0]


def kernel(y_pred: np.ndarray, y_target: np.ndarray) -> np.ndarray:
    in_maps, n_total = make_inputs(y_pred, y_target)
    nc = build_nc(IMG_PER_CORE)
    res = run_bass_kernel_spmd(nc, in_maps, list(range(N_CORES)))
    return np.asarray(combine([res.results[c] for c in range(N_CORES)], n_total))


# revision 5
# speedup vs baseline: 1.0914x; 1.0446x over previous
"""AdptWeightBCEDiceLoss Trainium2 kernel, v2 (w-pool-first redesign).

Full inputs y_pred/y_target [32,1,512,512] f32 -> scalar f32 loss.
8 cores x 4 images, data parallel.

Per-core pipeline (half-image units j=0..7, image i=j//2), with t5 = 5t:
  DMA:  s1_i = t5 padded rows [128,4,560] bf16 ([32:544) data), Pf_i bf16
  DVE:  fused 4-row scan -> sc_i (31-wide running w-boxsum, bf16)
  PE:   h-pool band matmul (bb = band/961) + (-I)*t5 accumulate
        -> D'_j = 5*avgpool - t5 in psum f32 [128,2,512]
  ACT:  q_j = |D'_j| -> SBUF bf16, accum -> sq_j  (abs, sigmoid-set filler)
  ACT:  F_j = sigmoid(-Pf_j), accum -> sF_j
  DVE:  ABS_AMR custom op: u_j = (|D'_j|+1)*t5, accum -> su5_j
  GpS:  qF_j = q_j * F_j
  PE:   colsum matmuls (ones stationary) over qF_j -> per-image psum row
  DVE:  TTR custom op: (u_j*F_j) accum -> sx5_j
  PE:   trace matmuls diag(Pf_i^T t5_i) -> zpsA/zpsB per image
  ACT:  phase 2: Ln(F) pairs, accum -> global sum ln F; psum evacs (Copy)
Host combines in float64:
  A = N + sq, sv = sF + colsum(qF), B = (su5-sx5)/5, C = A - sv + su5/5,
  bce = (-sum lnF - sum(x*t5)/5) / (32*N), w_iou = 1-(B+1+eps)/(C-B+1+eps),
  loss = mean(bce*~1 + w_iou)   [w_bce ~= bce; eps-correction negligible]
"""

import numpy as np

import concourse.bacc as bacc
import concourse.bass as bass
import concourse.tile as tile
from concourse import mybir
from concourse.bass_utils import run_bass_kernel_spmd

F32 = mybir.dt.float32
BF16 = mybir.dt.bfloat16
F8 = mybir.dt.float8e4

H = W = 512
RB = 4
KPOOL = 31
PADL = 32            # left pad (31 zeros needed; 32 keeps 4B alignment)
SROW = 560           # padded row: [0:32) zeros | [32:544) data | [544:560) zeros
NPIX = H * W
N_CORES = 8
IMG_PER_CORE = 4
NHALF = 2 * IMG_PER_CORE
SMOOTH = 1e-8
QSCALE = 1.0 / (KPOOL * KPOOL)


def _register_op(name, spec):
    import concourse.dve_ops as DO
    from concourse.dve_spec import lower, _has_src1
    from concourse.dve_uop import DveOpSpec

    for op in DO.OPS:
        if op.name == name:
            return op
    probe = DO.DveOp(name, spec, subdim=False, uops_sha={})
    DO.OPS.append(probe)
    DO.CUSTOM_DVE_SPECS[name] = spec
    DO._SUB_OPCODE_FOR_NAME[name] = DO._CUSTOM_DVE_ROW_BASE + len(DO.OPS) - 1
    shas = {}
    for ver in ("v3", "v4"):
        r = DveOpSpec(
            name=name,
            opcode=DO.get_dve_sub_opcode(name),
            uops=lower(spec, ver=ver),
            rd1_en=_has_src1(spec),
        )
        shas[ver] = r.sha(ver)
    final = DO.DveOp(name, spec, subdim=False, uops_sha=shas)
    DO.OPS[-1] = final
    return final


def register_custom_ops():
    """Register ABS_AFFINE_MUL_REDUCE: out=(|in0|*s0+s1)*in1, accum=sum(out).
    Runtime registration per the documented dve_ops authoring flow."""
    from concourse.dve_spec import Spec, Src0, Src1, C0, C1, Zero, maxx
    from operator import add as _add

    def ref(in0, in1, s0, s1, imm2):
        b = ((np.abs(in0.astype(np.float32)) * s0 + s1) * in1).astype(np.float32)
        return b, b.reshape(b.shape[0], -1).sum(axis=-1, keepdims=True)

    return _register_op(
        "ABS_AFFINE_MUL_REDUCE_ANT",
        Spec(body=(maxx(Src0, Zero - Src0) * C0 + C1) * Src1,
             accum=_add, accum_init=Zero, reference=ref),
    )


def register_boxsum_op():
    """out[t] = sum_{j<=t} (in0[j] - in1[j]) — running boxsum via a custom
    1-elem/cycle scan body (stock tensor_tensor_scan runs at ~2.24 cy/elem)."""
    from concourse.dve_spec import Spec, Src0, Src1, AluOp, scan

    def ref(in0, in1, s0, s1, imm2):
        return np.cumsum(in0.astype(np.float32) - in1.astype(np.float32),
                         axis=-1).astype(np.float32)

    return _register_op(
        "BOXSUM_SCAN_ANT",
        Spec(body=scan(AluOp.ADD, Src0 - Src1), reference=ref),
    )


def band_matrix_blocks() -> np.ndarray:
    """Three distinct band blocks [128, 3*128]: 0=diag |pi-po|<=15,
    1=below (pi-po>=113), 2=above (po-pi>=113); all scaled by 1/961."""
    import ml_dtypes

    pi = np.arange(128)[:, None]
    po = np.arange(128)[None, :]
    b0 = (np.abs(pi - po) <= 15).astype(np.float32)
    b1 = (pi - po >= 113).astype(np.float32)
    b2 = (po - pi >= 113).astype(np.float32)
    return np.ascontiguousarray(
        np.concatenate([b0, b1, b2], axis=1) * QSCALE
    ).astype(ml_dtypes.bfloat16)


def build_nc(n_img: int = IMG_PER_CORE) -> bacc.Bacc:
    register_custom_ops()
    register_boxsum_op()
    nc = bacc.Bacc("TRN2", target_bir_lowering=False, debug=False)
    pred_d = nc.dram_tensor("pb", [n_img, H, W], BF16, kind="ExternalInput")
    targ_d = nc.dram_tensor("tb5", [n_img, H, W], BF16, kind="ExternalInput")
    bb_d = nc.dram_tensor("bband", [128, 3 * 128], BF16, kind="ExternalInput")
    negI_d = nc.dram_tensor("negident", [128, 128], BF16, kind="ExternalInput")
    ones_d = nc.dram_tensor("ones1", [128, 1], BF16, kind="ExternalInput")
    acc_d = nc.dram_tensor("acc", [128, 34], F32, kind="ExternalOutput")
    cs_d = nc.dram_tensor("csq", [6, 512], F32, kind="ExternalOutput")
    zc_d = nc.dram_tensor("zcp", [128, 2, 128], BF16, kind="ExternalOutput")

    with tile.TileContext(nc) as tc:
        _body(tc, pred_d, targ_d, bb_d, negI_d, ones_d, acc_d, cs_d, zc_d, n_img)
    nc.compile()
    return nc


def _body(tc, pred_d, targ_d, bb_d, negI_d, ones_d, acc_d, cs_d, zc_d, n_img):
    from concourse.dve_ops import TENSOR_TENSOR_REDUCE
    ABS_AMR = register_custom_ops()
    BOXSUM = register_boxsum_op()

    nc = tc.nc
    ACTF = mybir.ActivationFunctionType
    MULT = mybir.AluOpType.mult
    ADD = mybir.AluOpType.add
    SUB = mybir.AluOpType.subtract
    nhalf = 2 * n_img

    with (
        tc.tile_pool(name="const", bufs=1) as constp,
        tc.tile_pool(name="sc", bufs=1) as scp,
        tc.tile_pool(name="q", bufs=1) as qp,
        tc.tile_pool(name="u", bufs=1) as up,
        tc.tile_pool(name="qf", bufs=1) as qfp,
        tc.tile_pool(name="xd", bufs=1) as xdp,
        tc.tile_pool(name="dps", bufs=1, space=bass.MemorySpace.PSUM) as dpsp,
        tc.tile_pool(name="zps", bufs=1, space=bass.MemorySpace.PSUM) as zpsp,
        tc.tile_pool(name="csps", bufs=1, space=bass.MemorySpace.PSUM) as csp,
    ):
        bb = constp.tile([128, 3 * 128], BF16)
        negI = constp.tile([128, 128], BF16)
        ones1 = constp.tile([128, 1], BF16)
        acc = constp.tile([128, 34], F32)
        s1bufs = [constp.tile([128, RB, SROW], BF16, tag=f"s1_{k}", name=f"s1_{k}")
                  for k in range(4)]
        Pf_t = [constp.tile([128, 2048], BF16, tag=f"pf_{k}", name=f"pf_{k}")
                for k in range(n_img)]
        Fbig = constp.tile([128, nhalf, 1024], BF16)
        csev = constp.tile([128, 1024], F32)
        zcp = constp.tile([128, 2, 128], BF16)

        # input DMA triggers FIRST: they must be the scalar queue's first
        # instructions so the HWDGE spin-up overlaps kernel boilerplate
        for _i in range(n_img):
            nc.scalar.dma_start(
                s1bufs[_i % 4][:, :, PADL:544],
                targ_d.ap()[_i].rearrange("(rb p) w -> p rb w", p=128),
            )
            nc.scalar.dma_start(
                Pf_t[_i][:].rearrange("p (rb w) -> p rb w", w=W),
                pred_d.ap()[_i].rearrange("(rb p) w -> p rb w", p=128),
            )

        # priming: custom-DVE uop table, gpsimd IRAM, sigmoid table
        zb = constp.tile([128, 1], F32)
        nc.vector.memset(zb[:], 0.0)
        pr0 = constp.tile([128, 1], F32)
        pr1 = constp.tile([128, 1], F32)
        nc.vector.affine_mul_reduce(pr0[:], pr1[:], zb[:], zb[:], 1.0, 0.0)
        prg = constp.tile([128, 1], F32)
        nc.gpsimd.tensor_tensor(prg[:], zb[:], zb[:], MULT)
        pra = constp.tile([128, 1], F32)
        nc.scalar.activation(pra[:], zb[:], ACTF.Sigmoid)

        for k in range(4):
            nc.vector.memset(s1bufs[k][:, :, 0:PADL], 0.0)
            nc.vector.memset(s1bufs[k][:, :, 544:SROW], 0.0)

        nc.sync.dma_start(bb[:], bb_d.ap())
        nc.sync.dma_start(negI[:], negI_d.ap())
        nc.sync.dma_start(ones1[:], ones_d.ap())

        cs_banks = [csp.tile([128, 512], F32, tag="cs0", name="cs0"),
                    csp.tile([128, 512], F32, tag="cs1", name="cs1")]
        zps = [zpsp.tile([128, 128], F32, tag="zA", name="zA"),
               zpsp.tile([128, 128], F32, tag="zB", name="zB")]

        sc_t, dps_t, q_t, u_t, qf_t = {}, {}, {}, {}, {}

        # distribute input DMAs across per-engine HW queues (4x bandwidth)
        s1_eng = [nc.scalar] * 4
        pf_eng = [nc.scalar] * 4

        def emit_dma(i):
            pass

        def emit_scan(i):
            s1 = s1bufs[i % 4]
            sc = scp.tile([128, RB, SROW], BF16, tag=f"sc_{i % 2}", name=f"sc_{i}")
            sc_t[i] = sc
            flat_in = s1[:].rearrange("p rb w -> p (rb w)")
            flat_out = sc[:].rearrange("p rb w -> p (rb w)")
            total = RB * SROW - (PADL + 1)
            nc.vector._custom_dve(
                BOXSUM, out=flat_out[:, 0:total],
                in0=flat_in[:, PADL:PADL + total],
                in1=flat_in[:, 1:1 + total],
            )

        def emit_hpool(i, h):
            s1 = s1bufs[i % 4]
            sc = sc_t[i]
            j = 2 * i + h
            rows = (2 * h, 2 * h + 1)
            dps = dpsp.tile([128, 2, 512], F32, tag=f"dps_{j % 2}", name=f"dps_{j}")
            dps_t[j] = dps
            for k, ro in enumerate(rows):
                ris = [r for r in (ro - 1, ro, ro + 1) if 0 <= r < RB]
                for m, ri in enumerate(ris):
                    blk = 0 if ri == ro else (1 if ri == ro - 1 else 2)
                    nc.tensor.matmul(
                        dps[:, k, :],
                        bb[:, blk * 128:(blk + 1) * 128],
                        sc[:, ri, 15:527],
                        start=(m == 0), stop=False,
                    )
                nc.tensor.matmul(
                    dps[:, k, :], negI[:], s1[:, ro, PADL:544],
                    start=False, stop=True,
                )

        def emit_trace(i):
            s1 = s1bufs[i % 4]
            Pf = Pf_t[i]
            for blk in range(16):
                rb, cb = divmod(blk, 4)
                gblk = 16 * i + blk
                nc.tensor.matmul(
                    zps[blk % 2][:],
                    Pf[:, blk * 128:(blk + 1) * 128],
                    s1[:, rb, PADL + cb * 128:PADL + (cb + 1) * 128],
                    start=(gblk < 2), stop=(gblk >= 16 * n_img - 2),
                )

        def emit_sigmoid(i, h):
            j = 2 * i + h
            nc.scalar.activation(Fbig[:, j, :], Pf_t[i][:, h * 1024:(h + 1) * 1024],
                                 ACTF.Sigmoid, scale=-1.0,
                                 accum_out=acc[:, 8 + j:9 + j])

        def emit_abs(i, h):
            j = 2 * i + h
            q = qp.tile([128, 2, 512], BF16, tag=f"q_{j % 2}", name=f"q_{j}")
            q_t[j] = q
            nc.scalar.activation(q[:], dps_t[j][:], ACTF.Abs,
                                 accum_out=acc[:, j:j + 1])

        def emit_u(i, h):
            j = 2 * i + h
            s1 = s1bufs[i % 4]
            rows = (2 * h, 2 * h + 1)
            t5v = s1[:, rows[0]:rows[0] + 2, PADL:544]
            u = up.tile([128, 2, 512], BF16, tag=f"u_{j % 2}", name=f"u_{j}")
            u_t[j] = u
            nc.vector._custom_dve(
                ABS_AMR, out=u[:], accum_out=acc[:, 16 + j:17 + j],
                in0=dps_t[j][:], in1=t5v, s0=1.0, s1=1.0,
            )

        def emit_qf(i, h):
            j = 2 * i + h
            qf = qfp.tile([128, 1024], BF16, tag=f"qf_{j}", name=f"qf_{j}")
            qf_t[j] = qf
            qflat = q_t[j][:].rearrange("p k w -> p (k w)")
            nc.gpsimd.tensor_tensor(qf[:], qflat, Fbig[:, j, :], MULT)

        def emit_x(i, h):
            j = 2 * i + h
            uflat = u_t[j][:].rearrange("p k w -> p (k w)")
            xd = xdp.tile([128, 1024], BF16, tag=f"xd_{j % 2}", name=f"xd_{j}")
            nc.vector._custom_dve(
                TENSOR_TENSOR_REDUCE, out=xd[:],
                accum_out=acc[:, 24 + j:25 + j],
                in0=uflat, in1=Fbig[:, j, :], s0=0.0, s1=1.0,
            )

        # ---- software-pipelined emission --------------------------------
        emit_scan(0)
        emit_trace(0)          # PE filler while scan_0 runs
        emit_sigmoid(0, 0)
        emit_sigmoid(0, 1)
        for i in range(n_img):
            if i + 1 < n_img:
                emit_scan(i + 1)           # DVE: scan ahead
                emit_sigmoid(i + 1, 0)     # ACT: sigmoid ahead (DMA-dep only)
                emit_sigmoid(i + 1, 1)
            for h in range(2):
                emit_hpool(i, h)
            for h in range(2):
                emit_abs(i, h)
                emit_u(i, h)
                emit_qf(i, h)
                emit_x(i, h)
            if i + 1 < n_img:
                emit_trace(i + 1)          # PE filler while scan_{i+2} runs

        # ---- tail phase -------------------------------------------------
        # colsum groups (qF all alive)
        for i in range(n_img):
            bank = cs_banks[0] if i < 3 else cs_banks[1]
            bp = 32 * i if i < 3 else 0
            for h in range(2):
                qf = qf_t[2 * i + h]
                for b in range(2):
                    nc.tensor.matmul(
                        bank[bp:bp + 1, :], ones1[:], qf[:, b * 512:(b + 1) * 512],
                        start=(h == 0 and b == 0), stop=(h == 1 and b == 1),
                    )
        # Ln over F pairs (one natural_log table load)
        lnscr = constp.tile([128, 4096], BF16)
        for pr in range(n_img // 2):
            nc.scalar.activation(
                lnscr[:],
                Fbig[:, 4 * pr:4 * pr + 4, :].rearrange("p a b -> p (a b)"),
                ACTF.Ln, accum_out=acc[:, 32 + pr:33 + pr],
            )
        # evacs (Copy: filler in every set)
        for ch in range(2):
            nc.scalar.activation(zcp[:, ch, :], zps[ch][:], ACTF.Copy)
        nc.scalar.activation(csev[:, 0:512], cs_banks[0][:], ACTF.Copy)
        nc.scalar.activation(csev[:, 512:1024], cs_banks[1][:], ACTF.Copy)

        nc.scalar.dma_start(acc_d.ap(), acc[:])
        for r, (p0, c0) in enumerate([(0, 0), (32, 0), (64, 0), (0, 512),
                                      (32, 512), (64, 512)]):
            nc.scalar.dma_start(cs_d.ap()[r:r + 1, :], csev[p0:p0 + 1, c0:c0 + 512])
        nc.scalar.dma_start(zc_d.ap(), zcp[:])


def combine(results, n_img_total):
    """results: list of dicts with acc [128,34], csq [128,1024], zcp [...]."""
    n_img = IMG_PER_CORE
    loss_terms = []
    g_total = 0.0
    for r in results:
        a = r["acc"].astype(np.float64)
        cs = r["csq"].astype(np.float64)
        zc = r["zcp"].astype(np.float64)   # [128, 2, 128] global chains
        sq_h = a[:, 0:8].sum(axis=0)       # per half
        sF_h = a[:, 8:16].sum(axis=0)
        su5_h = a[:, 16:24].sum(axis=0)
        sx5_h = a[:, 24:32].sum(axis=0)
        slnF = a[:, 32:34].sum()           # global over core
        g_total += -slnF
        szt5 = sum(np.trace(zc[:, ch, :]) for ch in range(2))
        g_total += -szt5 / 5.0
        for i in range(n_img):
            sq = sq_h[2 * i] + sq_h[2 * i + 1]
            sF = sF_h[2 * i] + sF_h[2 * i + 1]
            su5 = su5_h[2 * i] + su5_h[2 * i + 1]
            sx5 = sx5_h[2 * i] + sx5_h[2 * i + 1]
            row = cs[i]
            sqF = row.sum()
            A = NPIX + sq
            sv = sF + sqF
            B = (su5 - sx5) / 5.0
            C = A - sv + su5 / 5.0
            w_iou = 1.0 - (B + 1.0 + SMOOTH) / (C - B + 1.0 + SMOOTH)
            loss_terms.append((A, w_iou))
    bce = g_total / (n_img_total * NPIX)
    total = 0.0
    for A, w_iou in loss_terms:
        w_bce = (A * bce + SMOOTH) / (A + SMOOTH)
        total += w_bce + w_iou
    return np.float32(total / n_img_total)


def make_inputs(y_pred: np.ndarray, y_target: np.ndarray):
    import ml_dtypes

    pred = np.ascontiguousarray(np.asarray(y_pred, dtype=np.float32).reshape(-1, H, W))
    targ = np.ascontiguousarray(np.asarray(y_target, dtype=np.float32).reshape(-1, H, W))
    pb = pred.astype(ml_dtypes.bfloat16)
    tb5 = (5.0 * targ).astype(ml_dtypes.bfloat16)
    bb = band_matrix_blocks()
    negI = (-np.eye(128, dtype=np.float32)).astype(ml_dtypes.bfloat16)
    ones1 = np.ones((128, 1), dtype=np.float32).astype(ml_dtypes.bfloat16)
    in_maps = [
        {
            "pb": np.ascontiguousarray(pb[c * IMG_PER_CORE:(c + 1) * IMG_PER_CORE]),
            "tb5": np.ascontiguousarray(tb5[c * IMG_PER_CORE:(c + 1) * IMG_PER_CORE]),
            "bband": bb,
            "negident": negI,
            "ones1": ones1,
        }
        for c in range(N_CORES)
    ]
    return in_maps, pred.shape[0]


def kernel(y_pred: np.ndarray, y_target: np.ndarray) -> np.ndarray:
    in_maps, n_total = make_inputs(y_pred, y_target)
    nc = build_nc(IMG_PER_CORE)
    res = run_bass_kernel_spmd(nc, in_maps, list(range(N_CORES)))
    return np.asarray(combine([res.results[c] for c in range(N_CORES)], n_total))


# revision 6
# speedup vs baseline: 1.1109x; 1.0178x over previous
"""AdptWeightBCEDiceLoss Trainium2 kernel, v2 (w-pool-first redesign).

Full inputs y_pred/y_target [32,1,512,512] f32 -> scalar f32 loss.
8 cores x 4 images, data parallel.

Per-core pipeline (half-image units j=0..7, image i=j//2), with t5 = 5t:
  DMA:  s1_i = t5 padded rows [128,4,560] bf16 ([32:544) data), Pf_i bf16
  DVE:  fused 4-row scan -> sc_i (31-wide running w-boxsum, bf16)
  PE:   h-pool band matmul (bb = band/961) + (-I)*t5 accumulate
        -> D'_j = 5*avgpool - t5 in psum f32 [128,2,512]
  ACT:  q_j = |D'_j| -> SBUF bf16, accum -> sq_j  (abs, sigmoid-set filler)
  ACT:  F_j = sigmoid(-Pf_j), accum -> sF_j
  DVE:  ABS_AMR custom op: u_j = (|D'_j|+1)*t5, accum -> su5_j
  GpS:  qF_j = q_j * F_j
  PE:   colsum matmuls (ones stationary) over qF_j -> per-image psum row
  DVE:  TTR custom op: (u_j*F_j) accum -> sx5_j
  PE:   trace matmuls diag(Pf_i^T t5_i) -> zpsA/zpsB per image
  ACT:  phase 2: Ln(F) pairs, accum -> global sum ln F; psum evacs (Copy)
Host combines in float64:
  A = N + sq, sv = sF + colsum(qF), B = (su5-sx5)/5, C = A - sv + su5/5,
  bce = (-sum lnF - sum(x*t5)/5) / (32*N), w_iou = 1-(B+1+eps)/(C-B+1+eps),
  loss = mean(bce*~1 + w_iou)   [w_bce ~= bce; eps-correction negligible]
"""

import numpy as np

import concourse.bacc as bacc
import concourse.bass as bass
import concourse.tile as tile
from concourse import mybir
from concourse.bass_utils import run_bass_kernel_spmd

F32 = mybir.dt.float32
BF16 = mybir.dt.bfloat16
F8 = mybir.dt.float8e4

H = W = 512
RB = 4
KPOOL = 31
PADL = 32            # left pad (31 zeros needed; 32 keeps 4B alignment)
SROW = 560           # padded row: [0:32) zeros | [32:544) data | [544:560) zeros
NPIX = H * W
N_CORES = 8
IMG_PER_CORE = 4
NHALF = 2 * IMG_PER_CORE
SMOOTH = 1e-8
QSCALE = 1.0 / (KPOOL * KPOOL)


def _register_op(name, spec):
    import concourse.dve_ops as DO
    from concourse.dve_spec import lower, _has_src1
    from concourse.dve_uop import DveOpSpec

    for op in DO.OPS:
        if op.name == name:
            return op
    probe = DO.DveOp(name, spec, subdim=False, uops_sha={})
    DO.OPS.append(probe)
    DO.CUSTOM_DVE_SPECS[name] = spec
    DO._SUB_OPCODE_FOR_NAME[name] = DO._CUSTOM_DVE_ROW_BASE + len(DO.OPS) - 1
    shas = {}
    for ver in ("v3", "v4"):
        r = DveOpSpec(
            name=name,
            opcode=DO.get_dve_sub_opcode(name),
            uops=lower(spec, ver=ver),
            rd1_en=_has_src1(spec),
        )
        shas[ver] = r.sha(ver)
    final = DO.DveOp(name, spec, subdim=False, uops_sha=shas)
    DO.OPS[-1] = final
    return final


def register_custom_ops():
    """Register ABS_AFFINE_MUL_REDUCE: out=(|in0|*s0+s1)*in1, accum=sum(out).
    Runtime registration per the documented dve_ops authoring flow."""
    from concourse.dve_spec import Spec, Src0, Src1, C0, C1, Zero, maxx
    from operator import add as _add

    def ref(in0, in1, s0, s1, imm2):
        b = ((np.abs(in0.astype(np.float32)) * s0 + s1) * in1).astype(np.float32)
        return b, b.reshape(b.shape[0], -1).sum(axis=-1, keepdims=True)

    return _register_op(
        "ABS_AFFINE_MUL_REDUCE_ANT",
        Spec(body=(maxx(Src0, Zero - Src0) * C0 + C1) * Src1,
             accum=_add, accum_init=Zero, reference=ref),
    )


def register_boxsum_op():
    """out[t] = sum_{j<=t} (in0[j] - in1[j]) — running boxsum via a custom
    1-elem/cycle scan body (stock tensor_tensor_scan runs at ~2.24 cy/elem)."""
    from concourse.dve_spec import Spec, Src0, Src1, AluOp, scan

    def ref(in0, in1, s0, s1, imm2):
        return np.cumsum(in0.astype(np.float32) - in1.astype(np.float32),
                         axis=-1).astype(np.float32)

    return _register_op(
        "BOXSUM_SCAN_ANT",
        Spec(body=scan(AluOp.ADD, Src0 - Src1), reference=ref),
    )


def band_matrix_blocks() -> np.ndarray:
    """Three distinct band blocks [128, 3*128]: 0=diag |pi-po|<=15,
    1=below (pi-po>=113), 2=above (po-pi>=113); all scaled by 1/961."""
    import ml_dtypes

    pi = np.arange(128)[:, None]
    po = np.arange(128)[None, :]
    b0 = (np.abs(pi - po) <= 15).astype(np.float32)
    b1 = (pi - po >= 113).astype(np.float32)
    b2 = (po - pi >= 113).astype(np.float32)
    return np.ascontiguousarray(
        np.concatenate([b0, b1, b2], axis=1) * QSCALE
    ).astype(ml_dtypes.bfloat16)


def build_nc(n_img: int = IMG_PER_CORE) -> bacc.Bacc:
    register_custom_ops()
    register_boxsum_op()
    nc = bacc.Bacc("TRN2", target_bir_lowering=False, debug=False)
    pred_d = nc.dram_tensor("pb", [n_img, H, W], BF16, kind="ExternalInput")
    targ_d = nc.dram_tensor("tb5", [n_img, H, W], BF16, kind="ExternalInput")
    bb_d = nc.dram_tensor("bband", [128, 3 * 128], BF16, kind="ExternalInput")
    negI_d = nc.dram_tensor("negident", [128, 128], BF16, kind="ExternalInput")
    ones_d = nc.dram_tensor("ones1", [128, 1], BF16, kind="ExternalInput")
    acc_d = nc.dram_tensor("acc", [128, 34], F32, kind="ExternalOutput")
    cs_d = nc.dram_tensor("csq", [128, 1024], F32, kind="ExternalOutput")
    zc_d = nc.dram_tensor("zcp", [128, 2, 128], BF16, kind="ExternalOutput")

    with tile.TileContext(nc) as tc:
        _body(tc, pred_d, targ_d, bb_d, negI_d, ones_d, acc_d, cs_d, zc_d, n_img)
    nc.compile()
    return nc


def _body(tc, pred_d, targ_d, bb_d, negI_d, ones_d, acc_d, cs_d, zc_d, n_img):
    from concourse.dve_ops import TENSOR_TENSOR_REDUCE
    ABS_AMR = register_custom_ops()
    BOXSUM = register_boxsum_op()

    nc = tc.nc
    ACTF = mybir.ActivationFunctionType
    MULT = mybir.AluOpType.mult
    ADD = mybir.AluOpType.add
    SUB = mybir.AluOpType.subtract
    nhalf = 2 * n_img

    with (
        tc.tile_pool(name="const", bufs=1) as constp,
        tc.tile_pool(name="sc", bufs=1) as scp,
        tc.tile_pool(name="q", bufs=1) as qp,
        tc.tile_pool(name="u", bufs=1) as up,
        tc.tile_pool(name="qf", bufs=1) as qfp,
        tc.tile_pool(name="xd", bufs=1) as xdp,
        tc.tile_pool(name="dps", bufs=1, space=bass.MemorySpace.PSUM) as dpsp,
        tc.tile_pool(name="zps", bufs=1, space=bass.MemorySpace.PSUM) as zpsp,
        tc.tile_pool(name="csps", bufs=1, space=bass.MemorySpace.PSUM) as csp,
    ):
        bb = constp.tile([128, 3 * 128], BF16)
        negI = constp.tile([128, 128], BF16)
        ones1 = constp.tile([128, 1], BF16)
        acc = constp.tile([128, 34], F32)
        s1bufs = [constp.tile([128, RB, SROW], BF16, tag=f"s1_{k}", name=f"s1_{k}")
                  for k in range(4)]
        Pf_t = [constp.tile([128, 2048], BF16, tag=f"pf_{k}", name=f"pf_{k}")
                for k in range(n_img)]
        Fbig = constp.tile([128, nhalf, 1024], BF16)
        csev = constp.tile([128, 1024], F32)
        zcp = constp.tile([128, 2, 128], BF16)

        # input DMA triggers FIRST: they must be the scalar queue's first
        # instructions so the HWDGE spin-up overlaps kernel boilerplate
        for _i in range(n_img):
            nc.scalar.dma_start(
                s1bufs[_i % 4][:, :, PADL:544],
                targ_d.ap()[_i].rearrange("(rb p) w -> p rb w", p=128),
            )
            nc.scalar.dma_start(
                Pf_t[_i][:].rearrange("p (rb w) -> p rb w", w=W),
                pred_d.ap()[_i].rearrange("(rb p) w -> p rb w", p=128),
            )

        # priming: custom-DVE uop table, gpsimd IRAM, sigmoid table
        zb = constp.tile([128, 1], F32)
        nc.vector.memset(zb[:], 0.0)
        pr0 = constp.tile([128, 1], F32)
        pr1 = constp.tile([128, 1], F32)
        nc.vector.affine_mul_reduce(pr0[:], pr1[:], zb[:], zb[:], 1.0, 0.0)
        prg = constp.tile([128, 1], F32)
        nc.gpsimd.tensor_tensor(prg[:], zb[:], zb[:], MULT)
        pra = constp.tile([128, 1], F32)
        nc.scalar.activation(pra[:], zb[:], ACTF.Sigmoid)

        for k in range(4):
            nc.vector.memset(s1bufs[k][:, :, 0:PADL], 0.0)
            nc.vector.memset(s1bufs[k][:, :, 544:SROW], 0.0)

        nc.sync.dma_start(bb[:], bb_d.ap())
        nc.sync.dma_start(negI[:], negI_d.ap())
        nc.sync.dma_start(ones1[:], ones_d.ap())

        cs_banks = [csp.tile([128, 512], F32, tag="cs0", name="cs0"),
                    csp.tile([128, 512], F32, tag="cs1", name="cs1")]
        zps = [zpsp.tile([128, 128], F32, tag="zA", name="zA"),
               zpsp.tile([128, 128], F32, tag="zB", name="zB")]

        sc_t, dps_t, q_t, u_t, qf_t = {}, {}, {}, {}, {}

        # distribute input DMAs across per-engine HW queues (4x bandwidth)
        s1_eng = [nc.scalar] * 4
        pf_eng = [nc.scalar] * 4

        def emit_dma(i):
            pass

        def emit_scan(i):
            s1 = s1bufs[i % 4]
            sc = scp.tile([128, RB, SROW], BF16, tag=f"sc_{i % 2}", name=f"sc_{i}")
            sc_t[i] = sc
            flat_in = s1[:].rearrange("p rb w -> p (rb w)")
            flat_out = sc[:].rearrange("p rb w -> p (rb w)")
            total = RB * SROW - (PADL + 1)
            nc.vector._custom_dve(
                BOXSUM, out=flat_out[:, 0:total],
                in0=flat_in[:, PADL:PADL + total],
                in1=flat_in[:, 1:1 + total],
            )

        def emit_hpool(i, h):
            s1 = s1bufs[i % 4]
            sc = sc_t[i]
            j = 2 * i + h
            rows = (2 * h, 2 * h + 1)
            dps = dpsp.tile([128, 2, 512], F32, tag=f"dps_{j % 2}", name=f"dps_{j}")
            dps_t[j] = dps
            for k, ro in enumerate(rows):
                ris = [r for r in (ro - 1, ro, ro + 1) if 0 <= r < RB]
                for m, ri in enumerate(ris):
                    blk = 0 if ri == ro else (1 if ri == ro - 1 else 2)
                    nc.tensor.matmul(
                        dps[:, k, :],
                        bb[:, blk * 128:(blk + 1) * 128],
                        sc[:, ri, 15:527],
                        start=(m == 0), stop=False,
                    )
                nc.tensor.matmul(
                    dps[:, k, :], negI[:], s1[:, ro, PADL:544],
                    start=False, stop=True,
                )

        def emit_trace(i):
            s1 = s1bufs[i % 4]
            Pf = Pf_t[i]
            for blk in range(16):
                rb, cb = divmod(blk, 4)
                gblk = 16 * i + blk
                nc.tensor.matmul(
                    zps[blk % 2][:],
                    Pf[:, blk * 128:(blk + 1) * 128],
                    s1[:, rb, PADL + cb * 128:PADL + (cb + 1) * 128],
                    start=(gblk < 2), stop=(gblk >= 16 * n_img - 2),
                )

        def emit_sigmoid(i):
            fv = Fbig[:, 2 * i:2 * i + 2, :].rearrange("p a b -> p (a b)")
            nc.scalar.activation(fv, Pf_t[i][:], ACTF.Sigmoid, scale=-1.0,
                                 accum_out=acc[:, 8 + i:9 + i])

        def emit_abs(i, h):
            j = 2 * i + h
            q = qp.tile([128, 2, 512], BF16, tag=f"q_{j % 2}", name=f"q_{j}")
            q_t[j] = q
            nc.scalar.activation(q[:], dps_t[j][:], ACTF.Abs,
                                 accum_out=acc[:, j:j + 1])

        def emit_u(i, h):
            j = 2 * i + h
            s1 = s1bufs[i % 4]
            rows = (2 * h, 2 * h + 1)
            t5v = s1[:, rows[0]:rows[0] + 2, PADL:544]
            u = up.tile([128, 2, 512], BF16, tag=f"u_{j % 2}", name=f"u_{j}")
            u_t[j] = u
            nc.vector._custom_dve(
                ABS_AMR, out=u[:], accum_out=acc[:, 16 + j:17 + j],
                in0=dps_t[j][:], in1=t5v, s0=1.0, s1=1.0,
            )

        def emit_qf(i, h):
            j = 2 * i + h
            qf = qfp.tile([128, 1024], BF16, tag=f"qf_{j}", name=f"qf_{j}")
            qf_t[j] = qf
            qflat = q_t[j][:].rearrange("p k w -> p (k w)")
            nc.gpsimd.tensor_tensor(qf[:], qflat, Fbig[:, j, :], MULT)

        def emit_x(i, h):
            j = 2 * i + h
            uflat = u_t[j][:].rearrange("p k w -> p (k w)")
            xd = xdp.tile([128, 1024], BF16, tag=f"xd_{j % 2}", name=f"xd_{j}")
            nc.vector._custom_dve(
                TENSOR_TENSOR_REDUCE, out=xd[:],
                accum_out=acc[:, 24 + j:25 + j],
                in0=uflat, in1=Fbig[:, j, :], s0=0.0, s1=1.0,
            )

        # ---- software-pipelined emission --------------------------------
        emit_scan(0)
        emit_trace(0)          # PE filler while scan_0 runs
        emit_sigmoid(0)
        for i in range(n_img):
            if i + 1 < n_img:
                emit_scan(i + 1)           # DVE: scan ahead
                emit_sigmoid(i + 1)        # ACT: sigmoid ahead (DMA-dep only)
            for h in range(2):
                emit_hpool(i, h)
            for h in range(2):
                emit_abs(i, h)
                emit_u(i, h)
                emit_qf(i, h)
                emit_x(i, h)
            if i + 1 < n_img:
                emit_trace(i + 1)          # PE filler while scan_{i+2} runs

        # ---- tail phase -------------------------------------------------
        # colsum groups (qF all alive)
        for i in range(n_img):
            bank = cs_banks[0] if i < 3 else cs_banks[1]
            bp = 32 * i if i < 3 else 0
            for h in range(2):
                qf = qf_t[2 * i + h]
                for b in range(2):
                    nc.tensor.matmul(
                        bank[bp:bp + 1, :], ones1[:], qf[:, b * 512:(b + 1) * 512],
                        start=(h == 0 and b == 0), stop=(h == 1 and b == 1),
                    )
        # Ln over F pairs (one natural_log table load)
        lnscr = constp.tile([128, 4096], BF16)
        for pr in range(n_img // 2):
            nc.scalar.activation(
                lnscr[:],
                Fbig[:, 4 * pr:4 * pr + 4, :].rearrange("p a b -> p (a b)"),
                ACTF.Ln, accum_out=acc[:, 32 + pr:33 + pr],
            )
        # evacs (Copy: filler in every set)
        for ch in range(2):
            nc.scalar.activation(zcp[:, ch, :], zps[ch][:], ACTF.Copy)
        nc.scalar.activation(csev[:, 0:512], cs_banks[0][:], ACTF.Copy)
        nc.scalar.activation(csev[:, 512:1024], cs_banks[1][:], ACTF.Copy)

        nc.scalar.dma_start(acc_d.ap(), acc[:])
        nc.scalar.dma_start(cs_d.ap(), csev[:])
        nc.scalar.dma_start(zc_d.ap(), zcp[:])


def combine(results, n_img_total):
    """results: list of dicts with acc [128,34], csq [128,1024], zcp [...]."""
    n_img = IMG_PER_CORE
    loss_terms = []
    g_total = 0.0
    for r in results:
        a = r["acc"].astype(np.float64)
        cs = r["csq"].astype(np.float64)
        zc = r["zcp"].astype(np.float64)   # [128, 2, 128] global chains
        sq_h = a[:, 0:8].sum(axis=0)       # per half
        sF_i = a[:, 8:12].sum(axis=0)      # per image
        su5_h = a[:, 16:24].sum(axis=0)
        sx5_h = a[:, 24:32].sum(axis=0)
        slnF = a[:, 32:34].sum()           # global over core
        g_total += -slnF
        szt5 = sum(np.trace(zc[:, ch, :]) for ch in range(2))
        g_total += -szt5 / 5.0
        for i in range(n_img):
            sq = sq_h[2 * i] + sq_h[2 * i + 1]
            sF = sF_i[i]
            su5 = su5_h[2 * i] + su5_h[2 * i + 1]
            sx5 = sx5_h[2 * i] + sx5_h[2 * i + 1]
            row = cs[32 * i, 0:512] if i < 3 else cs[0, 512:1024]
            sqF = row.sum()
            A = NPIX + sq
            sv = sF + sqF
            B = (su5 - sx5) / 5.0
            C = A - sv + su5 / 5.0
            w_iou = 1.0 - (B + 1.0 + SMOOTH) / (C - B + 1.0 + SMOOTH)
            loss_terms.append((A, w_iou))
    bce = g_total / (n_img_total * NPIX)
    total = 0.0
    for A, w_iou in loss_terms:
        w_bce = (A * bce + SMOOTH) / (A + SMOOTH)
        total += w_bce + w_iou
    return np.float32(total / n_img_total)


def make_inputs(y_pred: np.ndarray, y_target: np.ndarray):
    import ml_dtypes

    pred = np.ascontiguousarray(np.asarray(y_pred, dtype=np.float32).reshape(-1, H, W))
    targ = np.ascontiguousarray(np.asarray(y_target, dtype=np.float32).reshape(-1, H, W))
    pb = pred.astype(ml_dtypes.bfloat16)
    tb5 = (5.0 * targ).astype(ml_dtypes.bfloat16)
    bb = band_matrix_blocks()
    negI = (-np.eye(128, dtype=np.float32)).astype(ml_dtypes.bfloat16)
    ones1 = np.ones((128, 1), dtype=np.float32).astype(ml_dtypes.bfloat16)
    in_maps = [
        {
            "pb": np.ascontiguousarray(pb[c * IMG_PER_CORE:(c + 1) * IMG_PER_CORE]),
            "tb5": np.ascontiguousarray(tb5[c * IMG_PER_CORE:(c + 1) * IMG_PER_CORE]),
            "bband": bb,
            "negident": negI,
            "ones1": ones1,
        }
        for c in range(N_CORES)
    ]
    return in_maps, pred.shape[0]


def kernel(y_pred: np.ndarray, y_target: np.ndarray) -> np.ndarray:
    in_maps, n_total = make_inputs(y_pred, y_target)
    nc = build_nc(IMG_PER_CORE)
    res = run_bass_kernel_spmd(nc, in_maps, list(range(N_CORES)))
    return np.asarray(combine([res.results[c] for c in range(N_CORES)], n_total))
